# revision 48
# baseline (speedup 1.0000x reference)
"""GQA kernel for Trainium2, sharded over 8 NeuronCores.

Problem: x[2,2048,2048] -> GQA(HQ=16 q-heads, HKV=4 kv-heads, D=128) -> out[2,2048,2048]
Sharding: core c = b*4 + h handles batch b and kv-head group h (4 q-heads).
Wq/Wk/Wv column-sharded per head group, Wo row-sharded; partial outputs
summed on host per batch.

Per-core kernel (bf16 matmul operands, fp32 PSUM accumulation), fully
software-pipelined so the PE never idles:
  0. DMA restructure: weights in 4-e-group tiles issued k/v-first on SP,
     x tiles as single 512KB DMAs rotated over the gpsimd/scalar/vector
     queues (e=0 as four 512-col tiles, e=1,2 split in halves) so the
     first K-proj matmul waits only on ~384KB. Eight warm-up matmuls on a
     memset tile run during the DMA ramp so HAM reaches K=8/8 before the
     real matmuls start (saves the 1.2GHz cold window).
  1. K/V projections e-outer (all 4 psum column blocks live) so each x
     tile is consumed right after its DMA lands.
  2. V transposed via PE into va[j][key,d] tiles, with block 0's
     Q-projection fully interleaved and block 1's first 10 steps after.
  3. Main loop over 8 blocks (g q-head, ib 1024-query half). Per block:
     scoresT[j,i] = kT_j^T @ qT[g] (2x512 psum), exp on ACT -> et bf16,
     AV flipped orientation: va_j stationary, et moving -> rawT[d,i]
     accumulated in psum (no transposes). Q-projection is pipelined TWO
     blocks ahead: block b's j=1..11 slots carry steps 10..31 of block
     b+1's projection (so the qT casts clear the DVE queue ~3us before
     the block ends) and j=12..15 carry steps 0..9 of block b+2's.
     Every j then has >=2 filler matmuls, keeping per-j PE time above
     the 1.09us exp, so ACT never paces the tail of a block.
     After AV15 the psum is copied raw to SBUF (releases the single avp
     psum buffer fast). Softmax denominator: DVE accumulates sum_j et_j
     (12 et buffers so the exp never WAR-waits on the adds), GPSIMD
     partition_all_reduce runs during the next block, and the
     reciprocal + normalize multiply are emitted one block later at
     j=11/j=14 (early enough to feed the consumers, late enough that
     the reciprocal's RAW on the 6.7us all-reduce can never
     head-of-line-block the DVE queue).
  4. Output projection split: nb01 pass reads only attnT[:, 0:1024]
     (eo 0-2 of it run inside blocks 6-7's filler slots) then nb23.
     PSUM->SBUF bf16 copies split between ACT and DVE, stores alternate
     the SP and gpsimd queues; the last two eo groups split copies and
     stores in 256-col pieces so the final drain is ~2us.
"""

import math

import numpy as np

B = 2
N = 2048
E = 2048
HQ = 16
G = 4
HKV = 4
D = 128
FQ = G * D  # 512 q-features per group
P = 128
NB = N // 512  # 4 moving-dim chunks
ET = E // P  # 16 contraction tiles
JT = N // P  # 16 key tiles
IB2 = N // 1024  # 2 query blocks of 1024
SCALE = 1.0 / math.sqrt(D)

_CACHE: dict = {}


def _build_program():
    import concourse.bacc as bacc
    import concourse.tile as tile
    from concourse import mybir
    from concourse.bass_isa import ReduceOp
    from concourse.masks import make_identity

    f32 = mybir.dt.float32
    bf16 = mybir.dt.bfloat16
    nc = bacc.Bacc("TRN2", target_bir_lowering=False)

    xT_d = nc.dram_tensor("xT", [ET, P, N], bf16, kind="ExternalInput")
    wqT_d = nc.dram_tensor("wqT", [P, ET, FQ], bf16, kind="ExternalInput")
    wkT_d = nc.dram_tensor("wkT", [P, ET, D], bf16, kind="ExternalInput")
    wvT_d = nc.dram_tensor("wvT", [P, ET, D], bf16, kind="ExternalInput")
    woT_d = nc.dram_tensor("woT", [P, G, N], bf16, kind="ExternalInput")
    outT_d = nc.dram_tensor("outT", [ET, P, N], bf16, kind="ExternalOutput")

    BLOCKS = [(g, ib) for ib in range(IB2) for g in range(G)]

    with tile.TileContext(nc) as tc:
        with tc.tile_pool(name="persist", bufs=1) as persist, \
             tc.tile_pool(name="w1", bufs=1) as w1:
            # ---- DMA issue schedule (before any compute helpers so the
            # issuing engines start immediately after the preamble) ----
            warm = persist.tile([P, 512], bf16, tag="warm")
            nc.vector.memset(warm[:], 0.0)

            wk4 = [w1.tile([P, 4, D], bf16, name=f"wk4_{i}", tag=f"wk4_{i}")
                   for i in range(4)]
            wv4 = [w1.tile([P, 4, D], bf16, name=f"wv4_{i}", tag=f"wv4_{i}")
                   for i in range(4)]
            wq4 = [w1.tile([P, 4, FQ], bf16, name=f"wq4_{i}", tag=f"wq4_{i}")
                   for i in range(4)]
            wo_sb = persist.tile([P, G, N], bf16, tag="wo_sb")

            # Schedule (phase 1 is HBM-bound: 11MB before the main loop at
            # ~358GB/s aggregate, ~145GB/s per queue). Weights interleave
            # with x tiles on SP so each wk/wv group lands just before its
            # e-range; early x tiles are chunked for latency, later ones are
            # single 512KB DMAs round-robined across the three queues.
            xts = [None] * ET
            xchunks = {}

            def _xt_chunks(e, engs, n=4):
                w = N // n
                cts = [w1.tile([P, w], bf16, name=f"xt{e}c{q}",
                               tag=f"xt{e}c{q}") for q in range(n)]
                for q in range(n):
                    engs[q].dma_start(
                        out=cts[q][:], in_=xT_d[e, :, q * w:(q + 1) * w]
                    )
                xchunks[e] = (cts, w)

            def _xt_full(e, eng):
                xt = w1.tile([P, N], bf16, name=f"xt{e}", tag=f"xt{e}")
                eng.dma_start(out=xt[:], in_=xT_d[e, :, :])
                xts[e] = xt

            # NB: a dma_start ISSUE blocks on its rotating semaphore (the
            # previous DMA on that sem must complete), so urgent DMAs must
            # never sit behind slow 512KB transfers on the same engine.
            nc.sync.dma_start(out=wk4[0][:], in_=wkT_d[:, 0:4, :])
            nc.sync.dma_start(out=wv4[0][:], in_=wvT_d[:, 0:4, :])
            # the first two e0 chunks ride SP right behind the 128KB weight
            # groups (the gpsimd/scalar queues have a slow first-DMA ramp)
            _xt_chunks(0, [nc.sync, nc.sync, nc.gpsimd, nc.scalar])
            _xt_chunks(1, [nc.gpsimd, nc.scalar], n=2)
            _xt_chunks(2, [nc.scalar, nc.gpsimd], n=2)
            _xt_full(5, nc.sync)
            for i in (1, 2):
                nc.sync.dma_start(out=wk4[i][:], in_=wkT_d[:, 4 * i:4 * i + 4, :])
                nc.sync.dma_start(out=wv4[i][:], in_=wvT_d[:, 4 * i:4 * i + 4, :])
            _xt_full(10, nc.sync)
            nc.sync.dma_start(out=wk4[3][:], in_=wkT_d[:, 12:16, :])
            nc.sync.dma_start(out=wv4[3][:], in_=wvT_d[:, 12:16, :])
            for e in (3, 6, 8, 11):
                _xt_full(e, nc.gpsimd)
            for e in (4, 7, 9, 13):
                _xt_full(e, nc.scalar)
            _xt_full(12, nc.sync)
            _xt_full(14, nc.sync)
            _xt_full(15, nc.sync)
            # wq on the gpsimd/scalar tails: arrives ~33-36us, before the
            # vT-phase Q-projection needs it (sync would deliver it ~40)
            nc.gpsimd.dma_start(out=wq4[0][:], in_=wqT_d[:, 0:4, :])
            nc.scalar.dma_start(out=wq4[1][:], in_=wqT_d[:, 4:8, :])
            nc.gpsimd.dma_start(out=wq4[2][:], in_=wqT_d[:, 8:12, :])
            nc.scalar.dma_start(out=wq4[3][:], in_=wqT_d[:, 12:16, :])
            nc.sync.dma_start(out=wo_sb[:], in_=woT_d[:, :, :])

            def xap(e, col, width=512):
                if e in xchunks:
                    cts, w = xchunks[e]
                    c = col // w
                    assert col % w == 0 or (col // w == (col + width - 1) // w)
                    if w == 512:
                        return cts[c][:]
                    return cts[c][:, col - c * w:col - c * w + width]
                return xts[e][:, col:col + width]

            # ---- PE warm-up: keep HAM busy during the DMA ramp ----
            with tc.tile_pool(name="wrm", bufs=1, space="PSUM") as wrm:
                wps = wrm.tile([P, 512], f32, tag="wps")
                for _ in range(8):
                    nc.tensor.matmul(
                        wps[:], warm[:, 0:128], warm[:],
                        start=True, stop=True,
                    )

            # pull the ACT exp-table load off the critical path (reads the
            # memset warm tile, so it runs right after the preamble)
            wact = persist.tile([P, 1], bf16, name="wact", tag="wact")
            nc.scalar.activation(
                wact[:], warm[:, 0:1],
                mybir.ActivationFunctionType.Exp, scale=0.001,
            )

            ident = persist.tile([P, P], bf16, tag="ident")
            make_identity(nc, ident)

            qT = [persist.tile([P, N], bf16, name=f"qT{f}", tag=f"qT{f}")
                  for f in range(G)]
            kT = persist.tile([P, N], bf16, tag="kT")
            va = persist.tile([P, JT, P], bf16, tag="va")
            # split per (g, ib): phase 3's nb01 pass must not depend on the
            # last block's normalize (deps are tile-granular)
            attnT = [[persist.tile([P, 1024], bf16, name=f"attnT{g}_{ib}",
                                   tag=f"attnT{g}_{ib}")
                      for ib in range(IB2)] for g in range(G)]
            vTs_t = w1.tile([P, N], bf16, tag="vTs")

            def vTs(col, width):
                return vTs_t[:, col:col + width]

            ones_col = persist.tile([P, 1], bf16, tag="ones_col")
            nc.vector.memset(ones_col[:], 1.0)
            ones_row = persist.tile([1, P], bf16, tag="ones_row")
            nc.vector.memset(ones_row[:], 1.0)

            # ---------- phase 1: K/V projections, e-outer ----------
            with tc.tile_pool(name="pkv", bufs=1, space="PSUM") as pkv:
                kvps = [pkv.tile([P, 1024], f32, name=f"kv{nb}", tag=f"kv{nb}")
                        for nb in range(NB)]
                for e in range(ET):
                    st = e == 0
                    sp = e == ET - 1
                    for nb in range(NB):
                        nc.tensor.matmul(
                            kvps[nb][:, 0:512],
                            wk4[e // 4][:, e % 4, :], xap(e, nb * 512),
                            start=st, stop=sp,
                        )
                    for nb in range(NB):
                        nc.tensor.matmul(
                            kvps[nb][:, 512:1024],
                            wv4[e // 4][:, e % 4, :], xap(e, nb * 512),
                            start=st, stop=sp,
                        )
                # per-nb cast pairs split DVE/ACT so each psum bank pair is
                # fully read early (the next phase's psum tiles WAR against
                # these reads)
                for nb in range(NB):
                    sl = slice(nb * 512, (nb + 1) * 512)
                    nc.vector.tensor_copy(vTs(nb * 512, 512),
                                          kvps[nb][:, 512:1024])
                    nc.scalar.copy(kT[:, sl], kvps[nb][:, 0:512])

            # Q-projection emitters: 32 matmul steps per block, pipelined
            # TWO blocks ahead (steps 0..9 land in block b-2, 10..31 in b-1)
            with tc.tile_pool(name="qpp", bufs=2, space="PSUM") as qpp:

                def make_qsteps(g, ib, cast_on_act=False):
                    steps = [(h2, e) for h2 in range(2) for e in range(ET)]
                    tiles = {}

                    def emit(k):
                        if k >= len(steps):
                            return
                        h2, e = steps[k]
                        col = ib * 1024 + h2 * 512
                        if e == 0:
                            tiles[h2] = qpp.tile(
                                [P, 512], f32, name="qp", tag="qp"
                            )
                        nc.tensor.matmul(
                            tiles[h2][:],
                            wq4[e // 4][:, e % 4, g * P:(g + 1) * P],
                            xap(e, col),
                            start=(e == 0), stop=(e == ET - 1),
                        )
                        if e == ET - 1:
                            # block 7's casts go to ACT: the DVE runs a few
                            # microseconds behind its emission point by the
                            # last block, and p3fill's psum tiles WAR
                            # against these casts (3.3us stall measured)
                            if cast_on_act:
                                nc.scalar.copy(
                                    qT[g][:, col:col + 512], tiles[h2][:]
                                )
                            else:
                                nc.vector.tensor_copy(
                                    qT[g][:, col:col + 512], tiles[h2][:]
                                )

                    return emit

                q_emit = [make_qsteps(*BLOCKS[b], cast_on_act=(b == 7))
                          for b in range(len(BLOCKS))]

                # ---- v transpose with block 0's Q-proj (3/j) and block
                # 1's first 10 steps (2/j from j=11) interleaved ----
                with tc.tile_pool(name="ptr0", bufs=2, space="PSUM") as ptr0:
                    k0 = 0
                    k1 = 0
                    for j in range(JT):
                        tp = ptr0.tile([P, P], bf16, tag="tp0")
                        nc.tensor.transpose(
                            tp[:], vTs(j * P, P), ident[:]
                        )
                        # on ACT: keeps the DVE free for block 0's qT casts,
                        # which gate the first scores matmul
                        nc.scalar.copy(va[:, j, :], tp[:])
                        if j <= 10:
                            for _ in range(3):
                                if k0 < 32:
                                    q_emit[0](k0)
                                    k0 += 1
                        else:
                            for _ in range(2):
                                if k1 < 10:
                                    q_emit[1](k1)
                                    k1 += 1

                # ---------- main loop ----------
                nrm_scope = tc.tile_pool(name="nrm", bufs=2)
                nrm = nrm_scope.__enter__()
                pending = None  # (g, ib, araw, den) awaiting recip+mult

                def flush_recip():
                    # stage 1 of the deferred normalize: reciprocal only,
                    # so the DVE burst is split and the accumulator adds
                    # are disturbed less
                    nonlocal pending
                    if pending is None or len(pending) == 5:
                        return
                    pg, pib, praw, pden = pending
                    # in-place on the den tile (saves an 8KB pool tag)
                    nc.vector.reciprocal_approx_fast(pden[:], pden[:])
                    pending = (pg, pib, praw, pden, True)

                def flush_pending():
                    # NB: must stay on DVE — gpsimd tensor ops contend for
                    # the shared DVE SBUF port (adds degrade 692->2452ns)
                    # and force a gpsimd library reload around all_reduce
                    nonlocal pending
                    if pending is None:
                        return
                    flush_recip()
                    pg, pib, praw, rec, _ = pending
                    nc.vector.tensor_mul(
                        attnT[pg][pib][:], praw[:], rec[:],
                    )
                    pending = None

                with tc.tile_pool(name="et", bufs=12) as etp, \
                     tc.tile_pool(name="ps", bufs=2, space="PSUM") as ps, \
                     tc.tile_pool(name="pav", bufs=1, space="PSUM") as pav:

                    def make_p3fill():
                        # blocks 6-7 have no next-block Q-projection left;
                        # fill their spare PE slots with the first three
                        # output projection units (they only need ib0 attnT),
                        # reusing the idle qpp psum buffers
                        state = {}

                        def emit(k):
                            eo, m = k // 10, k % 10
                            if eo >= 4:
                                return
                            if m < 8:
                                nbh, f = m // 4, m % 4
                                if f == 0:
                                    state[(eo, nbh)] = qpp.tile(
                                        [P, 512], f32, name="qp", tag="qp"
                                    )
                                nc.tensor.matmul(
                                    state[(eo, nbh)][:],
                                    wo_sb[:, f, eo * P:(eo + 1) * P],
                                    attnT[f][0][:, nbh * 512:
                                                 (nbh + 1) * 512],
                                    start=(f == 0), stop=(f == G - 1),
                                )
                            elif m in (8, 9):
                                nbh = m - 8
                                otf = nrm.tile([P, 512], bf16, name="otf",
                                               tag="otf")
                                nc.scalar.copy(
                                    otf[:], state[(eo, nbh)][:]
                                )
                                nc.sync.dma_start(
                                    out=outT_d[eo, :,
                                               nbh * 512:(nbh + 1) * 512],
                                    in_=otf[:],
                                )

                        return emit

                    p3fill = make_p3fill()

                    # per-block filler stream: steps 10..31 of block b+1's
                    # projection then steps 0..9 of block b+2's; blocks 6-7
                    # fall through to the p3fill units
                    def fill_list(b):
                        if b <= 5:
                            return ([(q_emit[b + 1], k) for k in range(10, 32)]
                                    + [(q_emit[b + 2], k) for k in range(10)])
                        if b == 6:
                            return ([(q_emit[7], k) for k in range(10, 32)]
                                    + [(p3fill, k) for k in range(8)])
                        # 32 steps: block 7's j=12..15 slots would otherwise
                        # run empty (852ns/j < the 1109ns exp -> ACT-paced)
                        return [(p3fill, k) for k in range(8, 40)]

                    for bi, (g, ib) in enumerate(BLOCKS):
                        i0 = ib * 1024
                        fills = fill_list(bi)

                        acc = nrm.tile([P, 1024], bf16, tag="acc")
                        araw = nrm.tile([P, 1024], bf16, tag="araw")
                        avp = pav.tile([P, 1024], f32, tag="avp")
                        ets = []
                        sps_l = []

                        def scores(j):
                            sps = ps.tile([P, 1024], f32, tag="sps")
                            for half in range(2):
                                nc.tensor.matmul(
                                    sps[:, half * 512:(half + 1) * 512],
                                    kT[:, j * P:(j + 1) * P],
                                    qT[g][:, i0 + half * 512:
                                           i0 + (half + 1) * 512],
                                    start=True, stop=True,
                                )
                            sps_l.append(sps)

                        def expo(j):
                            et = etp.tile([P, 1024], bf16, tag="et")
                            nc.scalar.activation(
                                et[:], sps_l[j][:],
                                mybir.ActivationFunctionType.Exp, scale=SCALE,
                            )
                            ets.append(et)
                            if j == 0:
                                nc.vector.tensor_copy(acc[:], et[:])
                            else:
                                nc.vector.tensor_add(acc[:], acc[:], et[:])

                        def av(j):
                            for half in range(2):
                                nc.tensor.matmul(
                                    avp[:, half * 512:(half + 1) * 512],
                                    va[:, j, :],
                                    ets[j][:, half * 512:(half + 1) * 512],
                                    start=(j == 0), stop=(j == JT - 1),
                                )

                        scores(0)
                        expo(0)
                        k = 0
                        nf = len(fills)
                        for j in range(1, JT):
                            scores(j)
                            expo(j)
                            av(j - 1)
                            if j == 11:
                                # previous block's normalize: the GPSIMD
                                # all-reduce starts only after the last acc
                                # add (~block start + DVE lag) and takes
                                # 6.7us, so the recip must sit late enough
                                # in the DVE queue to never head-of-line
                                # block it (j=8 measured a 6.6us DVE freeze)
                                flush_recip()
                            elif j == 14:
                                flush_pending()
                            if j <= 11:
                                take = 2
                            else:
                                take = (nf - k + (15 - j)) // (16 - j)
                            for _ in range(take):
                                if k < nf:
                                    em, kk = fills[k]
                                    em(kk)
                                    k += 1
                        av(JT - 1)
                        while k < nf:
                            em, kk = fills[k]
                            em(kk)
                            k += 1

                        # raw copy releases the single avp psum buffer fast;
                        # on ACT: it follows exp15 directly instead of
                        # queueing behind the DVE accumulator adds
                        nc.scalar.copy(araw[:], avp[:])
                        if bi == len(BLOCKS) - 1:
                            # final block: denominator handled on the PE,
                            # emitted later inside phase 3 so nothing here
                            # waits on the accumulator tail
                            last_fin = (g, ib, araw, acc)
                        else:
                            den = nrm.tile([P, 1024], f32, tag="den")
                            nc.gpsimd.partition_all_reduce(
                                den[:], acc[:], P, ReduceOp.add
                            )
                            pending = (g, ib, araw, den)

                    # ------ phase 3: output projection (nb01 then nb23) ------
                    # emitted inside the main pool scope, reusing the sps/et
                    # pool tags: opening fresh pools here inserts a drain
                    # against the GPSIMD all-reduce. The last block's flush
                    # is deferred past the first few eo groups (they only
                    # read ib0 attnT).
                    gi3 = 0
                    for half3 in range(2):
                        nbs = (0, 1) if half3 == 0 else (2, 3)
                        # eo 0-3 of the nb01 half ran inside blocks 6-7
                        for eo in range(4 if half3 == 0 else 0, ET):
                            gi3 += 1
                            if half3 == 0 and eo == 5:
                                # last block's denominator: PE ones-matmul
                                # -> [1,1024] psum; acc is long complete
                                lg, lib, lraw, lacc = last_fin
                                dps = [qpp.tile([P, 512], f32, name="qp",
                                                tag="qp") for _ in range(2)]
                                for h in range(2):
                                    nc.tensor.matmul(
                                        dps[h][0:1, :], ones_col[:],
                                        lacc[:, h * 512:(h + 1) * 512],
                                        start=True, stop=True,
                                    )
                                recb = nrm.tile([1, 1024], bf16, tag="recb")
                                recf = nrm.tile([1, 1024], f32, tag="recf")
                                for h in range(2):
                                    nc.vector.reciprocal_approx_fast(
                                        recf[:, h * 512:(h + 1) * 512],
                                        dps[h][0:1, :],
                                    )
                                nc.vector.tensor_copy(recb[:], recf[:])
                                last_fin = (lg, lib, lraw, recb)
                            if half3 == 1 and eo == 0:
                                # finalize the last block: broadcast the
                                # reciprocal across partitions via a PE
                                # ones-matmul and normalize into attnT
                                lg, lib, lraw, lrecb = last_fin
                                bcs = [qpp.tile([P, 512], f32, name="qp",
                                                tag="qp") for _ in range(2)]
                                for h in range(2):
                                    nc.tensor.matmul(
                                        bcs[h][:], ones_row[:],
                                        lrecb[:, h * 512:(h + 1) * 512],
                                        start=True, stop=True,
                                    )
                                for h in range(2):
                                    nc.vector.tensor_mul(
                                        attnT[lg][lib][:,
                                                       h * 512:(h + 1) * 512],
                                        lraw[:, h * 512:(h + 1) * 512],
                                        bcs[h][:],
                                    )
                            # every 3rd group borrows the avp psum banks
                            # (idle after the main loop): 3 rotating
                            # buffers instead of 2, so the group's first
                            # matmul stops WAR-waiting the trailing
                            # PSUM->SBUF copies of group n-2
                            if gi3 % 3 == 0:
                                ops_ = pav.tile([P, 1024], f32, name="avp",
                                                tag="avp")
                            else:
                                ops_ = ps.tile([P, 1024], f32, name="sps",
                                               tag="sps")
                            for f in range(G):
                                for k, nb in enumerate(nbs):
                                    nc.tensor.matmul(
                                        ops_[:, k * 512:(k + 1) * 512],
                                        wo_sb[:, f, eo * P:(eo + 1) * P],
                                        attnT[f][nb // 2][
                                            :, (nb % 2) * 512:
                                               (nb % 2 + 1) * 512],
                                        start=(f == 0), stop=(f == G - 1),
                                    )
                            last2 = half3 == 1 and eo >= ET - 2
                            if last2:
                                # split copies and stores in 256-col pieces
                                # with a separate tile per piece (a shared
                                # tile would serialize the ACT/DVE writes)
                                # so the final drain after the last matmul
                                # is short
                                for k, nb in enumerate(nbs):
                                    for q in range(2):
                                        sl_o = slice(k * 512 + q * 256,
                                                     k * 512 + (q + 1) * 256)
                                        sl_d = slice(nb * 512 + q * 256,
                                                     nb * 512 + (q + 1) * 256)
                                        pc = nrm.tile(
                                            [P, 256], bf16,
                                            name=f"otp{2 * k + q}",
                                            tag=f"otp{2 * k + q}",
                                        )
                                        if (k + q) % 2 == 0:
                                            nc.scalar.copy(
                                                pc[:], ops_[:, sl_o]
                                            )
                                            ie = nc.sync
                                        else:
                                            nc.vector.tensor_copy(
                                                pc[:], ops_[:, sl_o]
                                            )
                                            ie = nc.gpsimd
                                        ie.dma_start(
                                            out=outT_d[eo, :, sl_d],
                                            in_=pc[:],
                                        )
                            else:
                                ot = etp.tile([P, 1024], bf16, name="et",
                                              tag="et")
                                for k, nb in enumerate(nbs):
                                    use_scalar = (eo + k) % 2 == 0
                                    if use_scalar:
                                        nc.scalar.copy(
                                            ot[:, k * 512:(k + 1) * 512],
                                            ops_[:, k * 512:(k + 1) * 512],
                                        )
                                    else:
                                        nc.vector.tensor_copy(
                                            ot[:, k * 512:(k + 1) * 512],
                                            ops_[:, k * 512:(k + 1) * 512],
                                        )
                                    ie = nc.sync if (eo + k) % 2 == 0 \
                                        else nc.gpsimd
                                    ie.dma_start(
                                        out=outT_d[eo, :,
                                                   nb * 512:(nb + 1) * 512],
                                        in_=ot[:, k * 512:(k + 1) * 512],
                                    )
            nrm_scope.__exit__(None, None, None)
    nc.finalize()
    return nc


def _get_program():
    if "nc" not in _CACHE:
        _CACHE["nc"] = _build_program()
    return _CACHE["nc"]


def _make_in_maps(x, Wq, Wk, Wv, Wo):
    import ml_dtypes

    bf = ml_dtypes.bfloat16

    def wtile(w):  # [rows, E] -> [P, ET_rows, rows_per] tiled on partition
        r = w.shape[0]
        return np.ascontiguousarray(
            w.T.reshape(ET, P, r).transpose(1, 0, 2)
        ).astype(bf)

    xT = [
        np.ascontiguousarray(x[b].T).astype(bf).reshape(ET, P, N) for b in range(B)
    ]
    in_maps = []
    for c in range(8):
        b, h = c // HKV, c % HKV
        wo = Wo[:, h * FQ:(h + 1) * FQ].T  # [FQ, E]
        in_maps.append({
            "xT": xT[b],
            "wqT": wtile(Wq[h * FQ:(h + 1) * FQ, :]),
            "wkT": wtile(Wk[h * D:(h + 1) * D, :]),
            "wvT": wtile(Wv[h * D:(h + 1) * D, :]),
            "woT": np.ascontiguousarray(
                wo.reshape(G, P, N).transpose(1, 0, 2)
            ).astype(bf),
        })
    return in_maps


def run_spmd(in_maps, trace=False, **kw):
    from concourse.bass_utils import run_bass_kernel_spmd

    nc = _get_program()
    return run_bass_kernel_spmd(nc, in_maps, list(range(8)), trace=trace, **kw)


def kernel(x, Wq, Wk, Wv, Wo, next_token_only=0, **_ignored):
    x = np.asarray(x, dtype=np.float32)
    Wq = np.asarray(Wq, dtype=np.float32)
    Wk = np.asarray(Wk, dtype=np.float32)
    Wv = np.asarray(Wv, dtype=np.float32)
    Wo = np.asarray(Wo, dtype=np.float32)

    res = run_spmd(_make_in_maps(x, Wq, Wk, Wv, Wo))
    outs = [np.asarray(r["outT"], dtype=np.float32).reshape(E, N)
            for r in res.results]
    full = np.empty((B, N, E), np.float32)
    for b in range(B):
        acc = outs[b * HKV].copy()
        for h in range(1, HKV):
            acc += outs[b * HKV + h]
        full[b] = acc.T
    return full
